# revision 3
# baseline (speedup 1.0000x reference)
"""CrossModalAttention Trainium2 kernel (8 NeuronCores, SPMD).

Sharding: query-pixel parallel. Each core computes all (batch, direction,
head) attention for its 392-pixel query slice; K/V convs are replicated
(each core consumes the full inputs), so no device collectives are needed.
The fusion MLP runs per-core on its query slice; outputs are concatenated
on the host. The (tiny) attention-map scalar is reproduced on the host from
input means, exploiting linearity of conv1x1 w.r.t. the spatial mean.
"""

import sys

sys.path.insert(0, "/opt/trn_rl_repo")

import numpy as np

import concourse.bass as bass
import concourse.mybir as mybir
import concourse.tile as tile
from concourse.bass_utils import run_bass_kernel_spmd
from concourse.masks import make_identity
from concourse.vector_clock import ScopedClock, VectorClock
from concourse.tile_sem_assignment import N_PROCS

# ---------------------------------------------------------------- constants
B = 2
D = 256
HP = 56
N = HP * HP            # 3136 pixels
NH = 8
HD = 32
NCORES = 8
QS = N // NCORES       # 392 queries per core
QH = QS // 2           # 196 per half-block
SUB = QS // 4          # 98-query subtile (partition dim of V matmul)
KC = (N + 127) // 128  # 25 key chunks (24*128 + 64)
SCALE = float(1.0 / np.sqrt(HD))
BN_EPS = 1e-5
FP32 = mybir.dt.float32
BF16 = mybir.dt.bfloat16

_PROGRAM = None  # cached (nc) build


# ------------------------------------------------- walrus wait-cap workarounds
def _patched_drain_and_barrier(self, tick_clock, wait_clock):
    # This walrus build caps sync waits at 1 per instruction; stock Tile puts
    # the whole global clock on one Drain. Emit one drain per pending proc.
    nc = self.nc
    g = tick_clock.global_clock
    for p in range(N_PROCS):
        if g[p] > 0:
            vc = VectorClock([g[q] if q == p else 0 for q in range(N_PROCS)])
            inst = nc.sync.drain()
            wait_clock.add_sem_waits(inst.ins, ScopedClock({None: vc}))
    nc.all_engine_barrier()
    assert self.sems is not None
    popped = nc._tile_sem_poison_stack.pop()
    assert popped is self._sem_poison
    nc.clear_and_free_semaphores(list(self.sems.allocated().values()))
    nc.all_engine_barrier()


tile.TileContext._drain_and_barrier = _patched_drain_and_barrier


def _split_excess_waits(nc, cap=1):
    import bass_rust

    for f in nc.m.functions:
        for bb in f.blocks:
            new_list = []
            changed = False
            for inst in bb.instructions:
                si = inst.sync_info
                waits = list(si.on_wait) if si and si.on_wait else []
                if len(waits) > cap:
                    changed = True
                    surplus, keep = waits[:-cap], waits[-cap:]
                    for w in surplus:
                        nop = nc.engines[inst.engine].nop(nofuse=True).ins
                        for f2 in nc.m.functions:
                            for bb2 in f2.blocks:
                                il = list(bb2.instructions)
                                if il and il[-1] is nop:
                                    il.pop()
                                    bb2.instructions = il
                        nop.sync_info = bass_rust.SyncInfo(
                            on_wait=[w], on_update=[]
                        )
                        new_list.append(nop)
                    inst.sync_info = bass_rust.SyncInfo(
                        on_wait=keep,
                        on_update=list(si.on_update) if si and si.on_update else [],
                    )
                new_list.append(inst)
            if changed:
                bb.instructions = new_list


# ---------------------------------------------------------------- program
def _build_program():
    nc = bass.Bass("TRN2", target_bir_lowering=False, debug=False)

    x_rgb = nc.dram_tensor("x_rgb", [B, D, N], FP32, kind="ExternalInput").ap()
    x_dep = nc.dram_tensor("x_dep", [B, D, N], FP32, kind="ExternalInput").ap()
    xq_rgb = nc.dram_tensor("xq_rgb", [B, D, QS], FP32, kind="ExternalInput").ap()
    xq_dep = nc.dram_tensor("xq_dep", [B, D, QS], FP32, kind="ExternalInput").ap()
    # packed weights, see host prep: [dir, 257, 256] etc (row 256 = bias)
    wq = nc.dram_tensor("wq", [2, D + 1, D], FP32, kind="ExternalInput").ap()
    wk = nc.dram_tensor("wk", [2, D + 1, D], FP32, kind="ExternalInput").ap()
    wv = nc.dram_tensor("wv", [2, D + 1, NH * (HD + 1)], FP32, kind="ExternalInput").ap()
    wf1 = nc.dram_tensor("wf1", [2 * D + 1, D], FP32, kind="ExternalInput").ap()
    wf2 = nc.dram_tensor("wf2", [D + 1, D], FP32, kind="ExternalInput").ap()

    out = nc.dram_tensor("out", [B, D, QS], FP32, kind="ExternalOutput").ap()

    xs = {0: x_rgb, 1: x_dep}      # modality index: 0=rgb 1=depth
    xqs = {0: xq_rgb, 1: xq_dep}
    V1W = HD + 1  # 33 cols per head in augmented V layout

    with tile.TileContext(nc) as tc:
        import contextlib

        with contextlib.ExitStack() as ctx:
            singles = ctx.enter_context(tc.tile_pool(name="singles", bufs=1))
            xpool = ctx.enter_context(tc.tile_pool(name="xpool", bufs=3))
            xqpool = ctx.enter_context(tc.tile_pool(name="xqpool", bufs=4))
            kpool = ctx.enter_context(tc.tile_pool(name="kpool", bufs=4))
            vpool1 = ctx.enter_context(tc.tile_pool(name="vpool1", bufs=2))
            qpool = ctx.enter_context(tc.tile_pool(name="qpool", bufs=4))
            attpool = ctx.enter_context(tc.tile_pool(name="attpool", bufs=2))
            fusTp = ctx.enter_context(tc.tile_pool(name="fusTp", bufs=8))
            fpool = ctx.enter_context(tc.tile_pool(name="fpool", bufs=8))
            hpool = ctx.enter_context(tc.tile_pool(name="hpool", bufs=2))
            opool = ctx.enter_context(tc.tile_pool(name="opool", bufs=2))
            rcpool = ctx.enter_context(tc.tile_pool(name="rcpool", bufs=4))
            # PSUM: scores 4 banks + V-accum 2 banks + misc 2 banks = 8
            spool = ctx.enter_context(tc.tile_pool(name="spool", bufs=1, space="PSUM"))
            vps_pool = ctx.enter_context(tc.tile_pool(name="vps", bufs=1, space="PSUM"))
            tpool = ctx.enter_context(tc.tile_pool(name="tpool", bufs=2, space="PSUM"))

            # ---- persistent small tiles
            ones = singles.tile([1, 512], BF16)
            nc.vector.memset(ones, 1.0)
            ident = singles.tile([128, 128], BF16)
            make_identity(nc, ident)

            _wctr = [0]

            def load_w(dram_ap, rows, cols):
                """Load packed weight [rows(<=513), cols] as bf16 chunk tiles
                plus the final bias row; returns (chunks, bias_row)."""
                nchunks = rows // 128
                chunks = []
                for c in range(nchunks):
                    _wctr[0] += 1
                    t = singles.tile([128, cols], BF16, name=f"wc{_wctr[0]}",
                                     tag=f"wc{_wctr[0]}")
                    nc.gpsimd.dma_start(out=t, in_=dram_ap[c * 128:(c + 1) * 128, :])
                    chunks.append(t)
                _wctr[0] += 1
                row = singles.tile([1, cols], BF16, name=f"wr{_wctr[0]}",
                                   tag=f"wr{_wctr[0]}")
                nc.gpsimd.dma_start(out=row, in_=dram_ap[rows - 1:rows, :])
                return chunks, row

            wq_sb = [load_w(wq[d], D + 1, D) for d in range(2)]
            wk_sb = [load_w(wk[d], D + 1, D) for d in range(2)]
            wv_sb = [load_w(wv[d], D + 1, NH * V1W) for d in range(2)]
            wf1_sb = load_w(wf1, 2 * D + 1, D)
            wf2_sb = load_w(wf2, D + 1, D)

            # fused^T staging per (b, subtile): [98 q, 512 ch] bf16
            # fused (conv layout) per (b, ch-tile): [128, 392] bf16
            for b in range(B):
                fusedT = [fusTp.tile([128, 2 * D], BF16, tag="fusT", name="fusedT") for _ in range(4)]
                fused = [fpool.tile([128, QS], BF16, tag="fus", name="fused") for _ in range(4)]

                for dr in range(2):  # 0: rgb attends depth; 1: depth attends rgb
                    qmod, kvmod = (0, 1) if dr == 0 else (1, 0)

                    # ---------------- convs for this (b, dir) ----------------
                    xk = []
                    for t in range(2):
                        xt = xpool.tile([128, N], BF16, tag="x", name="xk")
                        nc.gpsimd.dma_start(out=xt, in_=xs[kvmod][b, t * 128:(t + 1) * 128, :])
                        xk.append(xt)
                    xq_t = []
                    for t in range(2):
                        xt = xqpool.tile([128, QS], BF16, tag="xq", name="xqt")
                        nc.gpsimd.dma_start(out=xt, in_=xqs[qmod][b, t * 128:(t + 1) * 128, :])
                        xq_t.append(xt)

                    # K conv: kt[t] [128ch, N]
                    kt = [kpool.tile([128, N], BF16, tag="k", name="kt") for _ in range(2)]
                    for t in range(2):
                        for pc in range(8):
                            psl = tpool.tile([128, 512], FP32, tag="tp")
                            ps = psl[:, 0:QS]
                            pix = slice(pc * QS, (pc + 1) * QS)
                            nc.tensor.matmul(ps, wk_sb[dr][0][0][:, t * 128:(t + 1) * 128],
                                             xk[0][:, pix], start=True, stop=False)
                            nc.tensor.matmul(ps, wk_sb[dr][0][1][:, t * 128:(t + 1) * 128],
                                             xk[1][:, pix], start=False, stop=False)
                            nc.tensor.matmul(ps, wk_sb[dr][1][0:1, t * 128:(t + 1) * 128],
                                             ones[0:1, 0:QS], start=False, stop=True)
                            nc.vector.tensor_copy(kt[t][:, pix], ps)

                    # V^T conv (augmented, 33-interleaved + ones cols):
                    # v1t [128 pix, 25*264]
                    v1t = vpool1.tile([128, KC * NH * V1W], BF16, tag="v1t")
                    for pt in range(KC):
                        pw = 128 if pt < KC - 1 else N - 128 * (KC - 1)
                        psl = tpool.tile([128, 512], FP32, tag="tp")
                        ps = psl[0:pw, 0:NH * V1W]
                        pix = slice(pt * 128, pt * 128 + pw)
                        nc.tensor.matmul(ps, xk[0][:, pix], wv_sb[dr][0][0],
                                         start=True, stop=False)
                        nc.tensor.matmul(ps, xk[1][:, pix], wv_sb[dr][0][1],
                                         start=False, stop=False)
                        nc.tensor.matmul(ps, ones[0:1, 0:pw], wv_sb[dr][1],
                                         start=False, stop=True)
                        nc.vector.tensor_copy(
                            v1t[0:pw, pt * NH * V1W:(pt + 1) * NH * V1W], ps)

                    # Q conv (only this core's queries): qt[t] [128ch, QS]
                    qt = [qpool.tile([128, QS], BF16, tag="q", name="qt") for _ in range(2)]
                    for t in range(2):
                        psl = tpool.tile([128, 512], FP32, tag="tp")
                        ps = psl[:, 0:QS]
                        nc.tensor.matmul(ps, wq_sb[dr][0][0][:, t * 128:(t + 1) * 128],
                                         xq_t[0], start=True, stop=False)
                        nc.tensor.matmul(ps, wq_sb[dr][0][1][:, t * 128:(t + 1) * 128],
                                         xq_t[1], start=False, stop=False)
                        nc.tensor.matmul(ps, wq_sb[dr][1][0:1, t * 128:(t + 1) * 128],
                                         ones[0:1, 0:QS], start=False, stop=True)
                        nc.vector.tensor_copy(qt[t], ps)

                    # ---------------- attention ----------------
                    for hg in range(2):          # head groups of 4 (= ch tile)
                        for qh in range(2):      # query halves of 196
                            qsl = slice(qh * QH, (qh + 1) * QH)
                            att = attpool.tile([128, KC, 4, QH], BF16, tag="att")
                            sco = spool.tile([128, 4, 512], FP32, tag="sc")
                            for c in range(KC):
                                kw = 128 if c < KC - 1 else N - 128 * (KC - 1)
                                ksl = slice(c * 128, c * 128 + kw)
                                for h in range(4):
                                    nc.tensor.matmul(
                                        sco[0:kw, h, 0:QH],
                                        kt[hg][32 * h:32 * h + 32, ksl],
                                        qt[hg][32 * h:32 * h + 32, qsl],
                                        start=True, stop=True,
                                        tile_position=(32 * h, 0))
                                nc.scalar.activation(
                                    out=att[0:kw, c, :, :],
                                    in_=sco[0:kw, :, 0:QH],
                                    func=mybir.ActivationFunctionType.Exp,
                                    scale=SCALE)
                            # V phase: 4 passes of 2 accumulators
                            for pp in range(2):
                                for s in range(2):  # subtile inside this half
                                    s4 = qh * 2 + s
                                    vp = vps_pool.tile([128, 2, 512], FP32, tag="vp")
                                    for c in range(KC):
                                        kw = 128 if c < KC - 1 else N - 128 * (KC - 1)
                                        for i in range(2):
                                            h = pp * 2 + i
                                            hglob = hg * 4 + h
                                            nc.tensor.matmul(
                                                vp[0:SUB, i, 0:V1W],
                                                att[0:kw, c, h, s * SUB:(s + 1) * SUB],
                                                v1t[0:kw, c * NH * V1W + hglob * V1W:
                                                    c * NH * V1W + (hglob + 1) * V1W],
                                                start=(c == 0), stop=(c == KC - 1))
                                    rc = rcpool.tile([SUB, 2], FP32, tag="rc")
                                    nc.vector.reciprocal(rc, vp[0:SUB, :, HD:HD + 1])
                                    for i in range(2):
                                        h = pp * 2 + i
                                        ch0 = dr * D + (hg * 4 + h) * HD
                                        nc.vector.tensor_scalar_mul(
                                            fusedT[s4][0:SUB, ch0:ch0 + HD],
                                            vp[0:SUB, i, 0:HD],
                                            rc[:, i:i + 1])

                # ---------------- transpose fused^T -> fused ----------------
                for s4 in range(4):
                    for t in range(4):
                        pst = tpool.tile([128, 512], BF16, tag="tp")
                        nc.tensor.transpose(
                            pst[0:128, 0:SUB],
                            fusedT[s4][0:SUB, t * 128:(t + 1) * 128],
                            ident[0:SUB, 0:SUB])
                        nc.vector.tensor_copy(
                            fused[t][:, s4 * SUB:(s4 + 1) * SUB],
                            pst[0:128, 0:SUB])

                # ---------------- fusion MLP ----------------
                hn = [hpool.tile([128, QS], BF16, tag="hn", name="hn") for _ in range(2)]
                for t in range(2):
                    psl = tpool.tile([128, 512], FP32, tag="tp")
                    ps = psl[:, 0:QS]
                    for c in range(4):
                        nc.tensor.matmul(ps, wf1_sb[0][c][:, t * 128:(t + 1) * 128],
                                         fused[c], start=(c == 0), stop=False)
                    nc.tensor.matmul(ps, wf1_sb[1][0:1, t * 128:(t + 1) * 128],
                                     ones[0:1, 0:QS], start=False, stop=True)
                    nc.vector.tensor_scalar_max(hn[t], ps, 0.0)
                for t in range(2):
                    psl = tpool.tile([128, 512], FP32, tag="tp")
                    ps = psl[:, 0:QS]
                    for c in range(2):
                        nc.tensor.matmul(ps, wf2_sb[0][c][:, t * 128:(t + 1) * 128],
                                         hn[c], start=(c == 0), stop=False)
                    nc.tensor.matmul(ps, wf2_sb[1][0:1, t * 128:(t + 1) * 128],
                                     ones[0:1, 0:QS], start=False, stop=True)
                    ob = opool.tile([128, QS], FP32, tag="ob")
                    nc.vector.tensor_copy(ob, ps)
                    nc.sync.dma_start(out=out[b, t * 128:(t + 1) * 128, :], in_=ob)

    _split_excess_waits(nc)
    return nc


def _get_program():
    global _PROGRAM
    if _PROGRAM is None:
        _PROGRAM = _build_program()
    return _PROGRAM


# ---------------------------------------------------------------- host side
def _pack_weights(Wq, bq, Wk, bk, Wv, bv):
    wq_p = np.concatenate([Wq.T, bq[None, :]], axis=0).astype(np.float32)
    wk_p = np.concatenate([Wk.T, bk[None, :]], axis=0).astype(np.float32)
    wv_p = np.zeros((D + 1, NH * V1W_CONST), dtype=np.float32)
    for h in range(NH):
        wv_p[:D, h * V1W_CONST:h * V1W_CONST + HD] = Wv.T[:, h * HD:(h + 1) * HD]
        wv_p[D, h * V1W_CONST:h * V1W_CONST + HD] = bv[h * HD:(h + 1) * HD]
        wv_p[D, h * V1W_CONST + HD] = 1.0
    return wq_p, wk_p, wv_p


V1W_CONST = HD + 1


def kernel(rgb_features, depth_features, Wq_rd, bq_rd, Wk_d, bk_d, Wv_d, bv_d,
           Wq_dr, bq_dr, Wk_r, bk_r, Wv_r, bv_r, Wf1, bf1, gamma, beta, Wf2, bf2):
    nc = _get_program()

    x_rgb = np.ascontiguousarray(rgb_features.reshape(B, D, N), dtype=np.float32)
    x_dep = np.ascontiguousarray(depth_features.reshape(B, D, N), dtype=np.float32)

    wq_p = np.stack([
        np.concatenate([Wq_rd.T, bq_rd[None, :]], 0),
        np.concatenate([Wq_dr.T, bq_dr[None, :]], 0)]).astype(np.float32)
    wk_p = np.stack([
        np.concatenate([Wk_d.T, bk_d[None, :]], 0),
        np.concatenate([Wk_r.T, bk_r[None, :]], 0)]).astype(np.float32)

    def pack_v(Wv, bv):
        p = np.zeros((D + 1, NH * V1W_CONST), dtype=np.float32)
        for h in range(NH):
            p[:D, h * V1W_CONST:h * V1W_CONST + HD] = Wv.T[:, h * HD:(h + 1) * HD]
            p[D, h * V1W_CONST:h * V1W_CONST + HD] = bv[h * HD:(h + 1) * HD]
            p[D, h * V1W_CONST + HD] = 1.0
        return p

    wv_p = np.stack([pack_v(Wv_d, bv_d), pack_v(Wv_r, bv_r)])

    gp = (gamma / np.sqrt(1.0 + BN_EPS)).astype(np.float64)
    w1s = (Wf1.astype(np.float64) * gp[:, None])
    b1s = bf1.astype(np.float64) * gp + beta.astype(np.float64)
    wf1_p = np.concatenate([w1s.T, b1s[None, :]], 0).astype(np.float32)
    wf2_p = np.concatenate([Wf2.T, bf2[None, :]], 0).astype(np.float32)

    in_maps = []
    for c in range(NCORES):
        qsl = slice(c * QS, (c + 1) * QS)
        in_maps.append({
            "x_rgb": x_rgb,
            "x_dep": x_dep,
            "xq_rgb": np.ascontiguousarray(x_rgb[:, :, qsl]),
            "xq_dep": np.ascontiguousarray(x_dep[:, :, qsl]),
            "wq": wq_p, "wk": wk_p, "wv": wv_p,
            "wf1": wf1_p, "wf2": wf2_p,
        })

    res = run_bass_kernel_spmd(nc, in_maps, list(range(NCORES)))
    fused = np.concatenate([res.results[c]["out"] for c in range(NCORES)], axis=2)
    fused = fused.reshape(B, D, HP, HP)

    # attention map: Qg/Kg are conv1x1 of the spatial mean (linearity)
    xm_rgb = x_rgb.mean(axis=2)   # (B, D)
    xm_dep = x_dep.mean(axis=2)
    Qg = xm_rgb @ Wq_rd.T + bq_rd
    Kg = xm_dep @ Wk_d.T + bk_d
    att = (Qg * Kg).sum(axis=1) / NH    # (B,)
    amap = np.broadcast_to(
        att.astype(np.float32)[:, None, None], (B, HP, HP)).copy()
    return fused, amap
